# revision 17
# baseline (speedup 1.0000x reference)
"""Per-class mean (segment reduce) on 8 Trainium2 NeuronCores.

Algorithm
---------
out[c] = sum_{i: labels[i]==c} features[i] / max(count_c, 1),  C=1000, A=512.

Rows are split evenly across the 8 cores.  On the host each core's rows
are sorted by label and bucketed by class *window* w = c >> 7 (8 windows
of 128 classes = 1024 >= 1000 -> the 8 PSUM banks), window-major, padded
up to 128-row tile boundaries per window.

Features are quantized to fp8-e4m3 (1 B/elem) with *error feedback*
along each per-core (class, column) run: rows of one class are
consecutive after the sort, and each row stores q_i = fp8(x_i + e_{i-1})
with e_i the running residual.  The class sum then telescopes,
sum(q) = sum(x) - e_last, so the quantization noise does NOT accumulate
over the ~262 rows of a class; measured end-to-end error is ~6e-3
(vs 2.7e-2 for plain fp8 rounding).  The per-core tensor is stored
partition-major [128, T, 512]: row t*128+p lives at [p, t, :], so the
device streams it with plain contiguous DMA - no gather.  The first
chunks are small (4/4/8 tiles) so the matmul pipeline starts early;
steady-state chunks are 16 tiles (8 KB/partition).

Each 128-row tile is window-pure.  The PE does one mixed-dtype matmul
per tile (bf16 one-hot stationary x fp8 moving, 1 col/cycle, ~216 ns):

    psum_bank[w] += onehot_t.T @ q_tile              # fp32 PSUM

(fp8 DoubleRow pairs were tried and measured NO faster than two plain
matmuls on this hardware, so every tile uses the plain path.)  A tiny
[128, T] f32 slot table (slot = label & 127, -1 for padding) rides
along; the DVE builds each tile's one-hot [128 rows x 128 slots]
on-chip with a single tensor_scalar(is_equal) against an iota (fp8
output on DVE is a 6.5x slower path, so one-hots are bf16).

The one-hot weights are exact in bf16 and PSUM accumulates in fp32, so
the device sum equals sum(q) exactly.  Windows are contiguous in the
tile stream, so each PSUM bank closes in order and is copied + DMA'd
out (as bf16 partials, halving write-back traffic) overlapping the next
window's matmuls; the final window's copy is split across the
Activation and Vector engines and two DMA queues to shorten the
exposed tail.  The host adds the 8 per-core partials in f32 and divides
by the global counts (np.bincount), matching the reference order
(sum, then divide).

One SPMD program serves all 8 cores: the schedule depends only on the
cross-core max tile count per window; per-core data (features, slot
table, first one-hots) are inputs.  Compiled at call time, memoized
per schedule.
"""

import functools
import sys
import types

import numpy as np

N_CORES = 8
NUM_CLASSES = 1000
N_WINDOWS = 8          # class windows of 128 -> 8 PSUM banks
A_DIM = 512
K_TILES = 16           # steady-state 128-row tiles per DMA chunk
RAMP_CHUNKS = (4, 4, 8)  # first chunks, for fast pipeline start
N_BUFS = 4             # chunk double-buffering depth
OH_BUFS = 4            # one-hot chunk buffers
OH0_TILES = 16         # leading tiles whose one-hots come from the host
N_WARM = 20            # 128-col zero warm-up matmuls (PE p-state ramp)


def _install_axon_hooks_shim():
    """The slim agent image lacks antenv.axon_hooks; concourse imports it
    when tracing.  Provide a fallback so imports never fail."""
    if "antenv.axon_hooks" in sys.modules:
        return
    try:
        from trn_agent_boot.trn_boot import _ntff_profile_via_ctypes
        hook = _ntff_profile_via_ctypes("/opt/axon/libaxon_pjrt.so")
    except Exception:
        hook = None
    mod = types.ModuleType("antenv.axon_hooks")
    mod.get_axon_ntff_profile_hook = lambda: hook
    mod.set_axon_ntff_profile_hook = lambda h: None
    sys.modules["antenv.axon_hooks"] = mod
    # tracing tries to upload artifacts to shared storage; keep it local
    try:
        import concourse.bass_utils as _bu
        _bu.upload_artifacts = lambda tmpdir: tmpdir
    except Exception:
        pass


def _chunks(T):
    """Chunk start/size list: small ramp chunks, then K_TILES."""
    out, c0 = [], 0
    for r in RAMP_CHUNKS:
        if c0 + r > T:
            break
        out.append((c0, r))
        c0 += r
    while c0 < T:
        cc = min(K_TILES, T - c0)
        out.append((c0, cc))
        c0 += cc
    return out


@functools.lru_cache(maxsize=4)
def _build_program(w_tiles: tuple):
    """Trace + compile the SPMD Bass program for one schedule."""
    _install_axon_hooks_shim()
    import concourse.bacc as bacc
    import concourse.tile as tile
    from concourse import mybir

    F32 = mybir.dt.float32
    BF16 = mybir.dt.bfloat16
    FP8 = mybir.dt.float8e4
    T = sum(w_tiles)

    # window of each tile + first/last tile per window
    win_of, first_t, last_t = [], {}, {}
    for w in range(N_WINDOWS):
        for _ in range(w_tiles[w]):
            ti = len(win_of)
            win_of.append(w)
            first_t.setdefault(w, ti)
            last_t[w] = ti
    last_w = win_of[-1]

    nc = bacc.Bacc("TRN2", target_bir_lowering=False, debug=False)
    feat = nc.declare_dram_parameter("feat", [128, T * A_DIM], FP8,
                                     isOutput=False)
    slots = nc.declare_dram_parameter("slots", [128, T], F32,
                                      isOutput=False)
    oh0 = nc.declare_dram_parameter("oh0", [128, OH0_TILES * 128], BF16,
                                    isOutput=False)
    out_sums = nc.declare_dram_parameter("out_sums", [N_WINDOWS * 128, A_DIM],
                                         BF16, isOutput=True)
    featv = feat[:].rearrange("p (t e) -> p t e", e=A_DIM)

    with tile.TileContext(nc) as tc:
        with (
            tc.tile_pool(name="cst", bufs=1) as cst,
            tc.tile_pool(name="gb", bufs=N_BUFS) as gb_pool,
            tc.tile_pool(name="ohp", bufs=OH_BUFS) as oh_pool,
            tc.tile_pool(name="ps", bufs=1, space="PSUM") as ps_pool,
            tc.tile_pool(name="stg", bufs=1) as stg_pool,
        ):
            chunk_list = _chunks(T)
            gts = {}
            psum = [ps_pool.tile([128, A_DIM], F32, tag=f"ps_{w}",
                                 name=f"ps_{w}")
                    for w in range(N_WINDOWS)]

            # PE warm-up: cheap zero matmuls ahead of real work so the
            # p-state clock (1.2 GHz for the first ~3 us of activity) is
            # ramped to 2.4 GHz when real data lands.
            warm = cst.tile([128, 128], BF16, tag="warm")
            nc.gpsimd.memset(warm[:], 0)
            for _ in range(N_WARM):
                nc.tensor.matmul(psum[last_w][:, 0:128], warm[:], warm[:],
                                 start=True, stop=True,
                                 skip_group_check=True)

            # slot table + first one-hots ride the gpsimd-triggered queue
            # so the scheduler cannot defer them behind feature chunks;
            # the host one-hots decouple the first matmuls from the
            # late-booting Vector engine.
            slots_sb = cst.tile([128, T], F32, tag="slots_sb")
            nc.gpsimd.dma_start(slots_sb[:], slots[:])
            oh0_sb = cst.tile([128, OH0_TILES, 128], BF16, tag="oh0_sb")
            nc.gpsimd.dma_start(
                oh0_sb[:], oh0[:].rearrange("p (t j) -> p t j", j=128))
            c0, cc = chunk_list[0]
            gts[c0] = gb_pool.tile([128, K_TILES, A_DIM], FP8, tag="gt",
                                   name="gt")
            nc.sync.dma_start(gts[c0][:, :cc, :], featv[:, c0:c0 + cc, :])
            iota_b = cst.tile([128, 128], BF16, tag="iota_b")
            nc.gpsimd.iota(iota_b[:], pattern=[[1, 128]], base=0,
                           channel_multiplier=0,
                           allow_small_or_imprecise_dtypes=True)

            staging = stg_pool.tile([128, N_WINDOWS, A_DIM], BF16, tag="stg")

            def close_window(w):
                """PSUM bank w is final: move to DRAM (bf16), overlapping
                the remaining work.  The last window's copy is split across
                ACT+DVE and two DMA queues to shorten the tail."""
                if w != last_w:
                    nc.scalar.copy(staging[:, w, :], psum[w][:])
                    nc.gpsimd.dma_start(
                        out_sums[w * 128:(w + 1) * 128, :],
                        staging[:, w, :])
                    return
                h = A_DIM // 2
                nc.scalar.copy(staging[:, w, 0:h], psum[w][:, 0:h])
                nc.sync.dma_start(out_sums[w * 128:(w + 1) * 128, 0:h],
                                  staging[:, w, 0:h])
                nc.vector.tensor_scalar_add(staging[:, w, h:], psum[w][:, h:],
                                            0.0)
                nc.scalar.dma_start(out_sums[w * 128:(w + 1) * 128, h:],
                                    staging[:, w, h:])

            for c0, cc in chunk_list:
                if c0 not in gts:
                    gts[c0] = gb_pool.tile([128, K_TILES, A_DIM], FP8,
                                           tag="gt", name="gt")
                    nc.sync.dma_start(gts[c0][:, :cc, :],
                                      featv[:, c0:c0 + cc, :])
                gt = gts[c0]
                oh = oh_pool.tile([128, K_TILES, 128], BF16, tag="oh")
                for k in range(cc):
                    ti = c0 + k
                    w = win_of[ti]
                    if ti < OH0_TILES:
                        oh_sl = oh0_sb[:, ti, :]
                    else:
                        nc.vector.tensor_scalar(
                            oh[:, k, :], iota_b[:], slots_sb[:, ti:ti + 1],
                            None, op0=mybir.AluOpType.is_equal)
                        oh_sl = oh[:, k, :]
                    nc.tensor.matmul(psum[w][:], oh_sl, gt[:, k, :],
                                     start=(ti == first_t[w]),
                                     stop=(ti == last_t[w]),
                                     skip_group_check=True)
                    if ti == last_t[w]:
                        close_window(w)

    nc.compile()
    return nc


def _schedule(labels_all: np.ndarray):
    """Cross-core tile counts per window from labels only."""
    n = labels_all.shape[0]
    n_loc = n // N_CORES
    win = (labels_all.astype(np.int64) >> 7).reshape(N_CORES, n_loc)
    counts = np.stack([np.bincount(win[c], minlength=N_WINDOWS)
                       for c in range(N_CORES)])          # [cores, windows]
    w_tiles = tuple(int(-(-int(counts[:, w].max()) // 128))
                    for w in range(N_WINDOWS))
    return n_loc, w_tiles, win, counts


def _quantize_feedback(sorted_f32: np.ndarray, sorted_lab: np.ndarray, fp8):
    """fp8-e4m3 with error feedback along each equal-label run."""
    starts = np.flatnonzero(np.r_[True, np.diff(sorted_lab) != 0])
    lens = np.diff(np.r_[starts, len(sorted_lab)])
    q = np.empty_like(sorted_f32, dtype=fp8)
    carry = np.zeros((len(starts), sorted_f32.shape[1]), np.float32)
    for j in range(lens.max()):
        m = lens > j
        idx = starts[m] + j
        v = sorted_f32[idx] + carry[m]
        qj = v.astype(fp8)
        carry[m] = v - qj.astype(np.float32)
        q[idx] = qj
    return q


def make_inputs(features: np.ndarray, labels_np: np.ndarray):
    """Full host prep: schedule + per-core input tensors.

    Sharding: rows are globally label-sorted and dealt round-robin
    (sorted position i -> core i%8), so per-core per-window counts are
    balanced to +-1 row and the cross-core max (which sets the shared
    tile count T) carries almost no padding.
    """
    import ml_dtypes
    fp8 = ml_dtypes.float8_e4m3
    mld_bf16 = ml_dtypes.bfloat16

    lab = labels_np.astype(np.int64)
    gorder = np.argsort(lab, kind="stable")

    # per-core window tile counts (cross-core max, balanced by dealing)
    counts = np.stack([
        np.bincount(lab[gorder[c::N_CORES]] >> 7, minlength=N_WINDOWS)
        for c in range(N_CORES)])
    w_tiles = tuple(int(-(-int(counts[:, w].max()) // 128))
                    for w in range(N_WINDOWS))
    T = sum(w_tiles)
    off_el = np.concatenate([[0], np.cumsum(w_tiles)])[:N_WINDOWS] * 128

    in_maps = []
    for c in range(N_CORES):
        rows = gorder[c::N_CORES]
        slab = lab[rows]                       # sorted by construction
        sw = slab >> 7
        cnt = np.bincount(sw, minlength=N_WINDOWS)
        cum = np.concatenate([[0], np.cumsum(cnt)])
        rank = np.arange(len(rows)) - cum[sw]
        s = off_el[sw] + rank
        p, t = s % 128, s // 128

        f32 = features[rows].astype(np.float32, copy=False)
        q = _quantize_feedback(f32, slab, fp8)
        feat_host = np.zeros((128, T, A_DIM), dtype=fp8)
        feat_host[p, t] = q
        slots_host = np.full((128, T), -1.0, dtype=np.float32)
        slots_host[p, t] = (slab & 127).astype(np.float32)
        oh0_host = (slots_host[:, :OH0_TILES, None]
                    == np.arange(128, dtype=np.float32)[None, None, :]
                    ).astype(mld_bf16)
        in_maps.append({"feat": feat_host.reshape(128, T * A_DIM),
                        "slots": slots_host,
                        "oh0": oh0_host.reshape(128, OH0_TILES * 128)})
    return w_tiles, in_maps


last_run = None    # BassKernelResults of the most recent kernel() call
_last_state = None  # (nc, in_maps) of the most recent kernel() call


def rerun(n=1, trace=True):
    """Re-execute the last-compiled program on the same inputs; returns
    the list of exec_time_ns (requires a prior kernel() call)."""
    from concourse.bass_utils import run_bass_kernel_spmd
    nc, in_maps = _last_state
    times = []
    for _ in range(n):
        r = run_bass_kernel_spmd(nc, in_maps, list(range(N_CORES)),
                                 trace=trace)
        times.append(r.exec_time_ns)
    return times


def kernel(features: np.ndarray, labels: np.ndarray) -> np.ndarray:
    global last_run, _last_state
    _install_axon_hooks_shim()
    from concourse.bass_utils import run_bass_kernel_spmd

    features = np.asarray(features)
    labels_np = np.asarray(labels)
    n, a = features.shape
    assert a == A_DIM and n % N_CORES == 0

    w_tiles, in_maps = make_inputs(features, labels_np)
    nc = _build_program(w_tiles)

    res = run_bass_kernel_spmd(nc, in_maps, list(range(N_CORES)))
    last_run = res
    _last_state = (nc, in_maps)
    total = np.zeros((N_WINDOWS * 128, A_DIM), dtype=np.float32)
    for c in range(N_CORES):
        total += res.results[c]["out_sums"].astype(np.float32)
    for w in range(N_WINDOWS):      # windows with no rows anywhere: force 0
        if w_tiles[w] == 0:
            total[w * 128:(w + 1) * 128] = 0.0

    counts = np.bincount(labels_np.astype(np.int64), minlength=NUM_CLASSES)
    counts = np.maximum(counts[:NUM_CLASSES], 1).astype(np.float32)
    return total[:NUM_CLASSES] / counts[:, None]


# revision 18
# speedup vs baseline: 1.0550x; 1.0550x over previous
"""Per-class mean (segment reduce) on 8 Trainium2 NeuronCores.

Algorithm
---------
out[c] = sum_{i: labels[i]==c} features[i] / max(count_c, 1),  C=1000, A=512.

Rows are split evenly across the 8 cores.  On the host each core's rows
are sorted by label and bucketed by class *window* w = c >> 7 (8 windows
of 128 classes = 1024 >= 1000 -> the 8 PSUM banks), window-major, padded
up to 128-row tile boundaries per window.

Features are quantized to fp8-e4m3 (1 B/elem) with *error feedback*
along each per-core (class, column) run: rows of one class are
consecutive after the sort, and each row stores q_i = fp8(x_i + e_{i-1})
with e_i the running residual.  The class sum then telescopes,
sum(q) = sum(x) - e_last, so the quantization noise does NOT accumulate
over the ~262 rows of a class; measured end-to-end error is ~6e-3
(vs 2.7e-2 for plain fp8 rounding).  The per-core tensor is stored
partition-major [128, T, 512]: row t*128+p lives at [p, t, :], so the
device streams it with plain contiguous DMA - no gather.  The first
chunks are small (4/4/8 tiles) so the matmul pipeline starts early;
steady-state chunks are 16 tiles (8 KB/partition).

Each 128-row tile is window-pure.  The PE does one mixed-dtype matmul
per tile (bf16 one-hot stationary x fp8 moving, 1 col/cycle, ~216 ns):

    psum_bank[w] += onehot_t.T @ q_tile              # fp32 PSUM

(fp8 DoubleRow pairs were tried and measured NO faster than two plain
matmuls on this hardware, so every tile uses the plain path.)  A tiny
[128, T] f32 slot table (slot = label & 127, -1 for padding) rides
along; the DVE builds each tile's one-hot [128 rows x 128 slots]
on-chip with a single tensor_scalar(is_equal) against an iota (fp8
output on DVE is a 6.5x slower path, so one-hots are bf16).

The one-hot weights are exact in bf16 and PSUM accumulates in fp32, so
the device sum equals sum(q) exactly.  Windows are contiguous in the
tile stream, so each PSUM bank closes in order and is copied + DMA'd
out (as bf16 partials, halving write-back traffic) overlapping the next
window's matmuls; the final window's copy is split across the
Activation and Vector engines and two DMA queues to shorten the
exposed tail.  The host adds the 8 per-core partials in f32 and divides
by the global counts (np.bincount), matching the reference order
(sum, then divide).

One SPMD program serves all 8 cores: the schedule depends only on the
cross-core max tile count per window; per-core data (features, slot
table, first one-hots) are inputs.  Compiled at call time, memoized
per schedule.
"""

import functools
import sys
import types

import numpy as np

N_CORES = 8
NUM_CLASSES = 1000
N_WINDOWS = 8          # class windows of 128 -> 8 PSUM banks
A_DIM = 512
K_TILES = 16           # steady-state 128-row tiles per DMA chunk
RAMP_CHUNKS = (4, 4, 8)  # first chunks, for fast pipeline start
N_BUFS = 4             # chunk double-buffering depth
OH_BUFS = 4            # one-hot chunk buffers


def _install_axon_hooks_shim():
    """The slim agent image lacks antenv.axon_hooks; concourse imports it
    when tracing.  Provide a fallback so imports never fail."""
    if "antenv.axon_hooks" in sys.modules:
        return
    try:
        from trn_agent_boot.trn_boot import _ntff_profile_via_ctypes
        hook = _ntff_profile_via_ctypes("/opt/axon/libaxon_pjrt.so")
    except Exception:
        hook = None
    mod = types.ModuleType("antenv.axon_hooks")
    mod.get_axon_ntff_profile_hook = lambda: hook
    mod.set_axon_ntff_profile_hook = lambda h: None
    sys.modules["antenv.axon_hooks"] = mod
    # tracing tries to upload artifacts to shared storage; keep it local
    try:
        import concourse.bass_utils as _bu
        _bu.upload_artifacts = lambda tmpdir: tmpdir
    except Exception:
        pass


def _chunks(T):
    """Chunk start/size list: small ramp chunks, then K_TILES."""
    out, c0 = [], 0
    for r in RAMP_CHUNKS:
        if c0 + r > T:
            break
        out.append((c0, r))
        c0 += r
    while c0 < T:
        cc = min(K_TILES, T - c0)
        out.append((c0, cc))
        c0 += cc
    return out


@functools.lru_cache(maxsize=4)
def _build_program(w_tiles: tuple):
    """Trace + compile the SPMD Bass program for one schedule."""
    _install_axon_hooks_shim()
    import concourse.bacc as bacc
    import concourse.tile as tile
    from concourse import mybir

    F32 = mybir.dt.float32
    BF16 = mybir.dt.bfloat16
    FP8 = mybir.dt.float8e4
    T = sum(w_tiles)

    # window of each tile + first/last tile per window
    win_of, first_t, last_t = [], {}, {}
    for w in range(N_WINDOWS):
        for _ in range(w_tiles[w]):
            ti = len(win_of)
            win_of.append(w)
            first_t.setdefault(w, ti)
            last_t[w] = ti
    last_w = win_of[-1]

    nc = bacc.Bacc("TRN2", target_bir_lowering=False, debug=False)
    feat = nc.declare_dram_parameter("feat", [128, T * A_DIM], FP8,
                                     isOutput=False)
    slots = nc.declare_dram_parameter("slots", [128, T], F32,
                                      isOutput=False)
    out_sums = nc.declare_dram_parameter("out_sums", [N_WINDOWS * 128, A_DIM],
                                         BF16, isOutput=True)
    featv = feat[:].rearrange("p (t e) -> p t e", e=A_DIM)

    with tile.TileContext(nc) as tc:
        with (
            tc.tile_pool(name="cst", bufs=1) as cst,
            tc.tile_pool(name="gb", bufs=N_BUFS) as gb_pool,
            tc.tile_pool(name="ohp", bufs=OH_BUFS) as oh_pool,
            tc.tile_pool(name="ps", bufs=1, space="PSUM") as ps_pool,
            tc.tile_pool(name="stg", bufs=1) as stg_pool,
        ):
            chunk_list = _chunks(T)
            gts = {}
            psum = [ps_pool.tile([128, A_DIM], F32, tag=f"ps_{w}",
                                 name=f"ps_{w}")
                    for w in range(N_WINDOWS)]

            # slot table rides the gpsimd-triggered queue so the scheduler
            # cannot defer it behind feature chunks on the sync queue
            slots_sb = cst.tile([128, T], F32, tag="slots_sb")
            nc.gpsimd.dma_start(slots_sb[:], slots[:])
            c0, cc = chunk_list[0]
            gts[c0] = gb_pool.tile([128, K_TILES, A_DIM], FP8, tag="gt",
                                   name="gt")
            nc.sync.dma_start(gts[c0][:, :cc, :], featv[:, c0:c0 + cc, :])
            iota_b = cst.tile([128, 128], BF16, tag="iota_b")
            nc.gpsimd.iota(iota_b[:], pattern=[[1, 128]], base=0,
                           channel_multiplier=0,
                           allow_small_or_imprecise_dtypes=True)

            staging = stg_pool.tile([128, N_WINDOWS, A_DIM], BF16, tag="stg")

            def close_window(w):
                """PSUM bank w is final: move to DRAM (bf16), overlapping
                the remaining work.  The last window's copy is split across
                ACT+DVE and two DMA queues to shorten the tail."""
                if w != last_w:
                    nc.scalar.copy(staging[:, w, :], psum[w][:])
                    nc.gpsimd.dma_start(
                        out_sums[w * 128:(w + 1) * 128, :],
                        staging[:, w, :])
                    return
                h = A_DIM // 2
                nc.scalar.copy(staging[:, w, 0:h], psum[w][:, 0:h])
                nc.sync.dma_start(out_sums[w * 128:(w + 1) * 128, 0:h],
                                  staging[:, w, 0:h])
                nc.vector.tensor_scalar_add(staging[:, w, h:], psum[w][:, h:],
                                            0.0)
                nc.scalar.dma_start(out_sums[w * 128:(w + 1) * 128, h:],
                                    staging[:, w, h:])

            for c0, cc in chunk_list:
                if c0 not in gts:
                    gts[c0] = gb_pool.tile([128, K_TILES, A_DIM], FP8,
                                           tag="gt", name="gt")
                    nc.sync.dma_start(gts[c0][:, :cc, :],
                                      featv[:, c0:c0 + cc, :])
                gt = gts[c0]
                oh = oh_pool.tile([128, K_TILES, 128], BF16, tag="oh")
                for k in range(cc):
                    ti = c0 + k
                    w = win_of[ti]
                    nc.vector.tensor_scalar(
                        oh[:, k, :], iota_b[:], slots_sb[:, ti:ti + 1],
                        None, op0=mybir.AluOpType.is_equal)
                    nc.tensor.matmul(psum[w][:], oh[:, k, :], gt[:, k, :],
                                     start=(ti == first_t[w]),
                                     stop=(ti == last_t[w]),
                                     skip_group_check=True)
                    if ti == last_t[w]:
                        close_window(w)

    nc.compile()
    return nc


def _schedule(labels_all: np.ndarray):
    """Cross-core tile counts per window from labels only."""
    n = labels_all.shape[0]
    n_loc = n // N_CORES
    win = (labels_all.astype(np.int64) >> 7).reshape(N_CORES, n_loc)
    counts = np.stack([np.bincount(win[c], minlength=N_WINDOWS)
                       for c in range(N_CORES)])          # [cores, windows]
    w_tiles = tuple(int(-(-int(counts[:, w].max()) // 128))
                    for w in range(N_WINDOWS))
    return n_loc, w_tiles, win, counts


def _quantize_feedback(sorted_f32: np.ndarray, sorted_lab: np.ndarray, fp8):
    """fp8-e4m3 with error feedback along each equal-label run."""
    starts = np.flatnonzero(np.r_[True, np.diff(sorted_lab) != 0])
    lens = np.diff(np.r_[starts, len(sorted_lab)])
    q = np.empty_like(sorted_f32, dtype=fp8)
    carry = np.zeros((len(starts), sorted_f32.shape[1]), np.float32)
    for j in range(lens.max()):
        m = lens > j
        idx = starts[m] + j
        v = sorted_f32[idx] + carry[m]
        qj = v.astype(fp8)
        carry[m] = v - qj.astype(np.float32)
        q[idx] = qj
    return q


def make_inputs(features: np.ndarray, labels_np: np.ndarray):
    """Full host prep: schedule + per-core input tensors.

    Sharding: rows are globally label-sorted and dealt round-robin
    (sorted position i -> core i%8), so per-core per-window counts are
    balanced to +-1 row and the cross-core max (which sets the shared
    tile count T) carries almost no padding.
    """
    import ml_dtypes
    fp8 = ml_dtypes.float8_e4m3

    lab = labels_np.astype(np.int64)
    gorder = np.argsort(lab, kind="stable")

    # per-core window tile counts (cross-core max, balanced by dealing)
    counts = np.stack([
        np.bincount(lab[gorder[c::N_CORES]] >> 7, minlength=N_WINDOWS)
        for c in range(N_CORES)])
    w_tiles = tuple(int(-(-int(counts[:, w].max()) // 128))
                    for w in range(N_WINDOWS))
    T = sum(w_tiles)
    off_el = np.concatenate([[0], np.cumsum(w_tiles)])[:N_WINDOWS] * 128

    in_maps = []
    for c in range(N_CORES):
        rows = gorder[c::N_CORES]
        slab = lab[rows]                       # sorted by construction
        sw = slab >> 7
        cnt = np.bincount(sw, minlength=N_WINDOWS)
        cum = np.concatenate([[0], np.cumsum(cnt)])
        rank = np.arange(len(rows)) - cum[sw]
        s = off_el[sw] + rank
        p, t = s % 128, s // 128

        f32 = features[rows].astype(np.float32, copy=False)
        q = _quantize_feedback(f32, slab, fp8)
        feat_host = np.zeros((128, T, A_DIM), dtype=fp8)
        feat_host[p, t] = q
        slots_host = np.full((128, T), -1.0, dtype=np.float32)
        slots_host[p, t] = (slab & 127).astype(np.float32)

        in_maps.append({"feat": feat_host.reshape(128, T * A_DIM),
                        "slots": slots_host})
    return w_tiles, in_maps


last_run = None    # BassKernelResults of the most recent kernel() call
_last_state = None  # (nc, in_maps) of the most recent kernel() call


def rerun(n=1, trace=True):
    """Re-execute the last-compiled program on the same inputs; returns
    the list of exec_time_ns (requires a prior kernel() call)."""
    from concourse.bass_utils import run_bass_kernel_spmd
    nc, in_maps = _last_state
    times = []
    for _ in range(n):
        r = run_bass_kernel_spmd(nc, in_maps, list(range(N_CORES)),
                                 trace=trace)
        times.append(r.exec_time_ns)
    return times


def kernel(features: np.ndarray, labels: np.ndarray) -> np.ndarray:
    global last_run, _last_state
    _install_axon_hooks_shim()
    from concourse.bass_utils import run_bass_kernel_spmd

    features = np.asarray(features)
    labels_np = np.asarray(labels)
    n, a = features.shape
    assert a == A_DIM and n % N_CORES == 0

    w_tiles, in_maps = make_inputs(features, labels_np)
    nc = _build_program(w_tiles)

    res = run_bass_kernel_spmd(nc, in_maps, list(range(N_CORES)))
    last_run = res
    _last_state = (nc, in_maps)
    total = np.zeros((N_WINDOWS * 128, A_DIM), dtype=np.float32)
    for c in range(N_CORES):
        total += res.results[c]["out_sums"].astype(np.float32)
    for w in range(N_WINDOWS):      # windows with no rows anywhere: force 0
        if w_tiles[w] == 0:
            total[w * 128:(w + 1) * 128] = 0.0

    counts = np.bincount(labels_np.astype(np.int64), minlength=NUM_CLASSES)
    counts = np.maximum(counts[:NUM_CLASSES], 1).astype(np.float32)
    return total[:NUM_CLASSES] / counts[:, None]
